# revision 1
# baseline (speedup 1.0000x reference)
"""AttentionPairBias Trainium2 kernel (8-core SPMD, sharded over the q axis).

Self-contained: hardcodes shapes B=1, N=768, DS=384, DP=128, H=16.

Per core (96 q-rows):
  - pair slice [96, 768, 128] f32 streamed in chunks [128k, 8q, 128dp],
    cast to fp16, xbar-transposed to [128dp, 8q, 128k].
  - PE computes per (q,k): xgw[k,16] + S1 (ones col) with the transposed
    tile as stationary, and S2 from the squared tile.
  - LN folds algebraically: bias = rs*xgw - (rs*mu)*sgw + cb.
  - Per-head QK^T -> scoresT[k,q] (+bias, exp with cb folded in),
    AV with ones-augmented V -> [q, dh|Z], normalized by 1/Z,
    transposed on PE, projected against Wo.
"""
import numpy as np

B, N, DS, DP, H = 1, 768, 384, 128, 16
DH = DS // H  # 24
NCORES = 8
QC = N // NCORES          # 96 q rows per core
NQ = 8                    # q rows per group
NGRP = QC // NQ           # 12 groups
NCH = N // 128            # 6 k chunks
LN_EPS = 1e-5
SCALING = float(DS) ** (-0.5)

_cached = {}


def _build_program(reps=1, ablate=frozenset()):
    import concourse.bass as bass
    import concourse.mybir as mybir
    import concourse.tile as tile
    import concourse.bacc as bacc

    F32 = mybir.dt.float32
    F16 = mybir.dt.float16
    AF = mybir.ActivationFunctionType
    OP = mybir.AluOpType

    nc = bacc.Bacc("TRN2", target_bir_lowering=False)

    pair = nc.dram_tensor("pair", [QC, N, DP], F32, kind="ExternalInput")
    sT = nc.dram_tensor("sT", [DS, N], F32, kind="ExternalInput")
    sTq = nc.dram_tensor("sTq", [DS, QC], F32, kind="ExternalInput")
    npadq = ((H + 2) // 3) * 96
    Wq = nc.dram_tensor("Wq", [DS, npadq], F32, kind="ExternalInput")  # pre-scaled, head-padded
    Wk = nc.dram_tensor("Wk", [DS, npadq], F32, kind="ExternalInput")  # head-padded
    Wv = nc.dram_tensor("Wv", [DS, DS], F32, kind="ExternalInput")
    Wo = nc.dram_tensor("Wo", [DS, DS], F32, kind="ExternalInput")
    gwA = nc.dram_tensor("gwA", [DP, H + 1], F16, kind="ExternalInput")
    sgw = nc.dram_tensor("sgw", [H], F32, kind="ExternalInput")
    cbv = nc.dram_tensor("cbv", [H], F32, kind="ExternalInput")
    id96 = nc.dram_tensor("id96", [QC, QC], F32, kind="ExternalInput")
    out = nc.dram_tensor("out", [QC, DS], F32, kind="ExternalOutput")

    with tile.TileContext(nc) as tc:
        with tc.tile_pool(name="persist", bufs=1) as pp, \
             tc.tile_pool(name="stage", bufs=3) as stp, \
             tc.tile_pool(name="xt", bufs=3) as xtp, \
             tc.tile_pool(name="work", bufs=2) as wkp, \
             tc.tile_pool(name="stats", bufs=3) as sp, \
             tc.tile_pool(name="avp", bufs=1, space="PSUM") as avpp, \
             tc.tile_pool(name="pjp", bufs=2, space="PSUM") as pjp, \
             tc.tile_pool(name="mixp", bufs=2, space="PSUM") as mixp, \
             tc.tile_pool(name="scp", bufs=1, space="PSUM") as scp:

            def _dummy(t):
                # ablation support: satisfy read-after-write tracking cheaply
                sl = t
                while len(sl.shape) > 2:
                    sl = sl[:, 0]
                nc.vector.memset(sl[0:1, 0:1], 0.0)

            # ---------------- prep: weights + single-side projections -------
            sT_sb = pp.tile([128, 3, N], F32)
            nc.sync.dma_start(out=sT_sb, in_=sT[:].rearrange("(c p) n -> p c n", p=128))
            sTq_sb = pp.tile([128, 3, QC], F32)
            nc.sync.dma_start(out=sTq_sb, in_=sTq[:].rearrange("(c p) n -> p c n", p=128))
            WqP_sb = pp.tile([128, 3, npadq], F32)
            nc.sync.dma_start(out=WqP_sb, in_=Wq[:].rearrange("(c p) n -> p c n", p=128))
            WkP_sb = pp.tile([128, 3, npadq], F32)
            nc.sync.dma_start(out=WkP_sb, in_=Wk[:].rearrange("(c p) n -> p c n", p=128))
            Wv_sb = pp.tile([128, 3, DS], F32)
            nc.sync.dma_start(out=Wv_sb, in_=Wv[:].rearrange("(c p) n -> p c n", p=128))
            WoH = pp.tile([DH, H, DS], F32)
            nc.sync.dma_start(out=WoH, in_=Wo[:].rearrange("(h r) e -> r h e", r=DH))
            gw_sb = pp.tile([DP, H + 1], F16)
            nc.sync.dma_start(out=gw_sb, in_=gwA[:])
            sgw_sb = pp.tile([128, H], F32)
            sgw_ap = sgw[:]
            nc.sync.dma_start(
                out=sgw_sb,
                in_=bass.AP(tensor=sgw_ap.tensor, offset=sgw_ap.offset,
                            ap=[[0, 128], list(sgw_ap.ap[0])]),
            )
            cb_sb = pp.tile([128, H], F32)
            cb_ap = cbv[:]
            nc.sync.dma_start(
                out=cb_sb,
                in_=bass.AP(tensor=cb_ap.tensor, offset=cb_ap.offset,
                            ap=[[0, 128], list(cb_ap.ap[0])]),
            )
            eps_sb = pp.tile([128, 1], F32)
            nc.vector.memset(eps_sb, LN_EPS)
            id_sb = pp.tile([QC, QC], F32)
            nc.sync.dma_start(out=id_sb, in_=id96[:])

            # v (native [k, h, dh] layout) with ones column at dh=24
            v_sb = pp.tile([128, NCH, H, DH + 1], F16)
            nc.vector.memset(v_sb, 1.0)
            for nchunk in range(NCH):
                v_ps = mixp.tile([128, DS], F32, tag="mix")
                for ic in range(3):
                    nc.tensor.matmul(
                        v_ps,
                        sT_sb[:, ic, nchunk * 128:(nchunk + 1) * 128],
                        Wv_sb[:, ic, :],
                        start=(ic == 0), stop=(ic == 2),
                    )
                nc.vector.tensor_copy(
                    out=v_sb[:, nchunk, :, 0:DH],
                    in_=v_ps.rearrange("p (h d) -> p h d", h=H),
                )

            # qT_h, kT_h per head (partition base 0 tiles).
            # WqP/WkP pack 3 heads per 96-col group (head at 32*j, cols
            # 24..31 zero) so per-head psum reads start at 0/32/64.
            qTh = []
            kTh = []
            for h in range(H):
                qTh.append(pp.tile([DH, QC], F16, name=f"qTh{h}", tag=f"qTh{h}"))
                kTh.append(pp.tile([DH, N], F16, name=f"kTh{h}", tag=f"kTh{h}"))
            nhalf = (N + 383) // 384  # 384-col halves of the k projection
            ngrp3 = (H + 2) // 3
            for hb in range(ngrp3):  # groups of 3 heads = 96 padded cols
                nh = min(3, H - hb * 3)
                q_ps = mixp.tile([96, QC], F32, tag="mix")
                for ic in range(3):
                    nc.tensor.matmul(
                        q_ps[0:32 * nh, :],
                        WqP_sb[:, ic, hb * 96:hb * 96 + 32 * nh], sTq_sb[:, ic, :],
                        start=(ic == 0), stop=(ic == 2),
                    )
                for hh in range(nh):
                    h = hb * 3 + hh
                    nc.vector.tensor_copy(out=qTh[h], in_=q_ps[32 * hh:32 * hh + DH, :])
                for j in range(nhalf):
                    jn = min(384, N - j * 384)
                    k_ps = mixp.tile([96, 512], F32, tag="mix")
                    for ic in range(3):
                        nc.tensor.matmul(
                            k_ps[0:32 * nh, 0:jn],
                            WkP_sb[:, ic, hb * 96:hb * 96 + 32 * nh],
                            sT_sb[:, ic, j * 384:j * 384 + jn],
                            start=(ic == 0), stop=(ic == 2),
                        )
                    for hh in range(nh):
                        h = hb * 3 + hh
                        nc.scalar.activation(
                            out=kTh[h][:, j * 384:j * 384 + jn],
                            in_=k_ps[32 * hh:32 * hh + DH, 0:jn],
                            func=AF.Copy, bias=0.0, scale=1.0,
                        )

            # AV accumulator psum: [96 q, 16 h, 32pad] (1 bank)
            av_ps = avpp.tile([QC, H, 32], F32)

            # ---------------- main loop ------------------------------------
            for _rep in range(reps):
              for c in range(NCH):
                  bias_c = wkp.tile([128, QC, H], F32, tag="bias")
                  if "stats" in ablate and "sadd" not in ablate:
                      _dummy(bias_c)
                  for g in range(NGRP):
                      x32 = stp.tile([128, NQ, DP], F32, tag="x32")
                      if "contig" in ablate:
                          flat = pair[:].rearrange("q k d -> (q k d)")
                          seg = flat[(c * NGRP + g) * 128 * NQ * DP:
                                     (c * NGRP + g + 1) * 128 * NQ * DP]
                          nc.sync.dma_start(
                              out=x32, in_=seg.rearrange("(p f) -> p f", p=128))
                      elif "ingest" not in ablate:
                          nc.sync.dma_start(
                              out=x32,
                              in_=pair[g * NQ:(g + 1) * NQ, c * 128:(c + 1) * 128, :]
                              .rearrange("q k d -> k q d"),
                          )
                      elif "cast" not in ablate:
                          _dummy(x32)
                      x16 = stp.tile([128, NQ, DP], F16, tag="x16")
                      if "cast" not in ablate:
                          nc.vector.tensor_copy(out=x16, in_=x32)
                      elif "xbar" not in ablate:
                          _dummy(x16)
                      xT = xtp.tile([DP, NQ, 128], F16, tag="xT")
                      if "xbar" not in ablate:
                          nc.sync.dma_start_transpose(xT, x16)
                      elif "sq" not in ablate or "pj" not in ablate:
                          _dummy(xT)
                      xT2 = xtp.tile([DP, NQ, 128], F16, tag="xT2")
                      if "sq" not in ablate:
                          nc.gpsimd.tensor_tensor(out=xT2, in0=xT, in1=xT, op=OP.mult)
                      elif "s2m" not in ablate:
                          _dummy(xT2)

                      pj = pjp.tile([128, NQ, 18], F32, tag="pj")
                      for q in range(NQ):
                          if "pj" not in ablate:
                              nc.tensor.matmul(
                                  pj[:, q, 0:H + 1], xT[:, q, :], gw_sb,
                                  start=True, stop=True,
                              )
                          if "s2m" not in ablate:
                              nc.tensor.matmul(
                                  pj[:, q, H + 1:H + 2], xT2[:, q, :], gw_sb[:, H:H + 1],
                                  start=True, stop=True,
                              )
                      if ("pj" in ablate and "s2m" in ablate
                              and "stats" not in ablate):
                          _dummy(pj)
                      if "stats" not in ablate:
                          # stats: evac S1/S2 then mu2, var, rs, trs (SBUF-only)
                          s12 = sp.tile([128, NQ, 2], F32, tag="s12")
                          nc.vector.tensor_copy(out=s12, in_=pj[:, :, H:H + 2])
                          mu2 = sp.tile([128, NQ], F32, tag="mu2")
                          nc.vector.scalar_tensor_tensor(
                              out=mu2, in0=s12[:, :, 0], scalar=1.0 / (DP * DP),
                              in1=s12[:, :, 0], op0=OP.mult, op1=OP.mult,
                          )
                          varp = sp.tile([128, NQ], F32, tag="varp")
                          nc.vector.scalar_tensor_tensor(
                              out=varp, in0=s12[:, :, 1], scalar=1.0 / DP,
                              in1=mu2, op0=OP.mult, op1=OP.subtract,
                          )
                          sd = sp.tile([128, NQ], F32, tag="sd")
                          nc.scalar.activation(out=sd, in_=varp, func=AF.Sqrt,
                                               bias=eps_sb, scale=1.0)
                          rs = sp.tile([128, NQ], F32, tag="rs")
                          nc.vector.reciprocal(out=rs, in_=sd)
                          trs = sp.tile([128, NQ], F32, tag="trs")
                          nc.vector.scalar_tensor_tensor(
                              out=trs, in0=s12[:, :, 0], scalar=1.0 / DP,
                              in1=rs, op0=OP.mult, op1=OP.mult,
                          )
                          # bias = rs*xgw - trs*sgw
                          rs_bc = bass.AP(tensor=rs.tensor, offset=rs.offset,
                                          ap=[rs.ap[0], rs.ap[1], [0, H]])
                          trs_bc = bass.AP(tensor=trs.tensor, offset=trs.offset,
                                           ap=[trs.ap[0], trs.ap[1], [0, H]])
                          sgw_bc = bass.AP(tensor=sgw_sb.tensor, offset=sgw_sb.offset,
                                           ap=[sgw_sb.ap[0], [0, NQ], sgw_sb.ap[1]])
                          rsx = sp.tile([128, NQ, H], F32, tag="rsx")
                          nc.vector.tensor_tensor(out=rsx, in0=pj[:, :, 0:H],
                                                  in1=rs_bc, op=OP.mult)
                          tsg = sp.tile([128, NQ, H], F32, tag="tsg")
                          nc.gpsimd.tensor_tensor(out=tsg, in0=trs_bc, in1=sgw_bc,
                                                  op=OP.mult)
                          nc.vector.tensor_tensor(
                              out=bias_c[:, g * NQ:(g + 1) * NQ, :],
                              in0=rsx, in1=tsg, op=OP.subtract,
                          )

                  # attention for this chunk (heads in half-batches of 8)
                  expT = wkp.tile([128, H, QC], F16, tag="expT")
                  for hb2 in range(2):
                      sc_ps = scp.tile([128, 8, 128], F32, tag="sc")
                      if "qk" in ablate and ("sadd" not in ablate or "exp" not in ablate):
                          _dummy(sc_ps)
                      for hh in range(8):
                          h = hb2 * 8 + hh
                          if "qk" not in ablate:
                              nc.tensor.matmul(
                                  sc_ps[:, hh, 0:QC],
                                  kTh[h][:, c * 128:(c + 1) * 128], qTh[h],
                                  start=True, stop=True,
                              )
                          if "sadd" not in ablate:
                              nc.vector.tensor_tensor(
                                  out=sc_ps[:, hh, 0:QC], in0=sc_ps[:, hh, 0:QC],
                                  in1=bias_c[:, :, h], op=OP.add,
                              )
                          if "exp" not in ablate:
                              nc.scalar.activation(
                                  out=expT[:, h, :], in_=sc_ps[:, hh, 0:QC],
                                  func=AF.Exp, bias=cb_sb[:, h:h + 1], scale=1.0,
                              )
                          if "exp" in ablate and "av" not in ablate and hh == 0:
                              _dummy(expT)
                          if "av" not in ablate:
                              nc.tensor.matmul(
                                  av_ps[:, h, 0:DH + 1], expT[:, h, :], v_sb[:, c, h, :],
                                  start=(c == 0 and h == 0),
                                  stop=(c == NCH - 1 and h == H - 1),
                              )

            # ---------------- final: normalize, transpose, out-proj --------
            if "av" in ablate:
                _dummy(av_ps)
            rz = pp.tile([QC, H], F32)
            nc.vector.reciprocal(out=rz, in_=av_ps[:, :, DH])
            attn = pp.tile([QC, H, DH], F32)
            for h in range(H):
                nc.vector.tensor_scalar_mul(
                    out=attn[:, h, :], in0=av_ps[:, h, 0:DH],
                    scalar1=rz[:, h:h + 1],
                )
            attnT = pp.tile([DH, H, QC], F32)
            for h in range(H):
                at_ps = mixp.tile([DH, QC], F32, tag="mix")
                nc.tensor.transpose(at_ps, attn[:, h, :], id_sb)
                nc.vector.tensor_copy(out=attnT[:, h, :], in_=at_ps)
            fin_ps = scp.tile([QC, DS], F32, tag="sc")
            for h in range(H):
                nc.tensor.matmul(
                    fin_ps, attnT[:, h, :], WoH[:, h, :],
                    start=(h == 0), stop=(h == H - 1),
                )
            out_sb = pp.tile([QC, DS], F32)
            nc.scalar.activation(out=out_sb, in_=fin_ps, func=AF.Copy,
                                 bias=0.0, scale=1.0)
            nc.sync.dma_start(out=out[:], in_=out_sb)

    nc.compile()
    return nc


def _host_prep(single_repr, pair_repr, Wq, Wk, Wv, ln_gamma, ln_beta, Wb, Wo):
    single = np.asarray(single_repr[0], dtype=np.float32)      # [N, DS]
    sT = np.ascontiguousarray(single.T)                        # [DS, N]
    gw = (np.asarray(ln_gamma, np.float32)[:, None] * np.asarray(Wb, np.float32))
    gwA = np.concatenate([gw, np.ones((DP, 1), np.float32)], axis=1).astype(np.float16)
    sgw = gw.sum(axis=0).astype(np.float32)                    # [H]
    cb = (np.asarray(ln_beta, np.float32) @ np.asarray(Wb, np.float32)).astype(np.float32)
    Wq_s = (np.asarray(Wq, np.float32) * SCALING).astype(np.float32)

    def _head_pad(W):
        # [DS, H*DH] -> [DS, ngrp3*96]: 3 heads per group, each on 32 cols
        ngrp3 = (H + 2) // 3
        out = np.zeros((DS, ngrp3 * 96), np.float32)
        for h in range(H):
            g, j = divmod(h, 3)
            out[:, g * 96 + j * 32: g * 96 + j * 32 + DH] = W[:, h * DH:(h + 1) * DH]
        return out

    base = {
        "sT": np.ascontiguousarray(sT),
        "Wq": np.ascontiguousarray(_head_pad(Wq_s)),
        "Wk": np.ascontiguousarray(_head_pad(np.asarray(Wk, np.float32))),
        "Wv": np.ascontiguousarray(np.asarray(Wv, np.float32)),
        "Wo": np.ascontiguousarray(np.asarray(Wo, np.float32)),
        "gwA": np.ascontiguousarray(gwA),
        "sgw": sgw,
        "cbv": cb,
        "id96": np.eye(QC, dtype=np.float32),
    }
    in_maps = []
    for i in range(NCORES):
        q0 = i * QC
        m = dict(base)
        m["pair"] = np.ascontiguousarray(np.asarray(pair_repr[0, q0:q0 + QC], np.float32))
        m["sTq"] = np.ascontiguousarray(sT[:, q0:q0 + QC])
        in_maps.append(m)
    return in_maps, cb


def kernel(single_repr, pair_repr, Wq, Wk, Wv, ln_gamma, ln_beta, Wb, Wo):
    from concourse.bass_utils import run_bass_kernel_spmd

    in_maps, _cb = _host_prep(single_repr, pair_repr, Wq, Wk, Wv,
                              ln_gamma, ln_beta, Wb, Wo)
    if "nc" not in _cached:
        _cached["nc"] = _build_program()
    nc = _cached["nc"]
    res = run_bass_kernel_spmd(nc, in_maps, core_ids=list(range(NCORES)))
    outs = [res.results[i]["out"] for i in range(NCORES)]
    return np.concatenate(outs, axis=0).reshape(B, N, DS).astype(np.float32)

